# revision 48
# baseline (speedup 1.0000x reference)
"""Linformer self-attention on 8 Trainium2 NeuronCores.

Problem (hardcoded shapes): x [4,4096,1024] f32; per batch:
  q = scale*(x@Wq); kv = x@Wkv; keys/values compressed 4096->256 via
  proj_k/proj_v; 16-head attention (dh=64, k=256); out @ Wproj + bproj.

Sharding: 8 cores = 4 batches x 2 head-groups (8 heads / 512 cols each).
Each core computes a partial [4096,1024] output (Wproj row-split); host
sums the pair and adds bias.

Numerics: phases A (x.T@projkv), B (Wq.T@xT) and E (oT.T@Wproj) run as
error-compensated fp8e4 DoubleRow matmuls: each operand X is split into
X_hi = fp8(X*s) and X_lo = fp8(X*s - X_hi) (power-of-2 s, exact to
undo).  X@W = sum_c [Xhi_c@Whi_c] + [Xhi_c@Wlo_c + Xlo_c@Whi_c],
dropping the O(eps^2) Xlo@Wlo term; DoubleRow packs 2 row-groups per
instruction at 0.5 cycles/row, so a K-128 chain costs 0.75x bf16 with
~13-bit-mantissa accuracy (better than bf16).  Per 128-row chunk the
lhsT-side tensor is stored [hi|lo] and the rhs-side [lo|hi], so the
main DR (hi_c,hi_c+1) and the correction DR ((hi_c,lo_c)x(lo_c,hi_c))
both slice out of one grid AP.  C, D, A2 and softmax stay bf16.

Phase order is B-first: the whole qT [512,4096] is computed before
phase A, because phase A is DMA-bound (x hi|lo 8MB + projkv 4MB vs
41us of PE) while phase B has DMA slack — its xt tiles stream at
1MB/5.1us of PE.  Phase A's x/kv tiles and the wk/wv/wproj weights
prefetch behind phase B / phase A, so the A stream and the merged
loop never starve.  The merged loop is then scores/E/D only, with
scores running one n-block ahead (D(nb) consumes exps that had a full
iteration of Act time to drain) and scores(0) interleaved into A2;
the oT hi/lo split runs on GPSIMD so DVE (o-normalize + out-evict)
stays under the PE per-iteration time.

Per-core dataflow (all matmuls use out = lhsT.T @ rhs, K<=128 partitions):
  B : qT[512,4096] = Wq_g.T @ xT                   (comp-fp8 DR)
  A : xcxvT[1024,512] = x.T @ [proj_k|proj_v]      (comp-fp8 DR)
  A2: kprojT[512,256] = Wk_g.T @ xcT ; vproj[256,512] = xvT.T @ Wv_g
  C : per (head,fc k-chunk): scoresT[128,512] -> exp (Act) -> pexp bf16
  S : per (n-chunk, head): sums[n,1] = pexp.T @ (1/so)  (N=1 matmuls)
  D : po[n, 8*64] = pexp.T @ vproj_h per head; normalize via DVE
      tensor_tensor with per-head recip broadcast -> o bf16 (so-scaled)
  T : oT via one batched DMA transpose per [128,512] tile; split into
      (hi,lo) fp8 on Act+DVE  (PE transposes for the final block)
  E : out[n,1024] = oT.T-chunks @ Wproj_g           (comp-fp8 DR)
"""

import os
import numpy as np

import concourse.bass as bass
import concourse.mybir as mybir
import concourse.tile as tile
from concourse import bacc
from concourse.bass_utils import run_bass_kernel_spmd

P = 128
N, D, K, DG, DH = 4096, 1024, 256, 512, 64
NB = 8                    # n-blocks of 512
HL = 8                    # heads per core
F32 = mybir.dt.float32

MMDT_NAME = os.environ.get("LINF_MMDT", "bfloat16")
MMDT = getattr(mybir.dt, MMDT_NAME)
Exp = mybir.ActivationFunctionType.Exp
F8 = mybir.dt.float8e4
DR = mybir.MatmulPerfMode.DoubleRow

# which phases run compensated-fp8 DoubleRow (subset of "abe")
COMP = os.environ.get("LINF_COMP", "abe")

# power-of-2 scales for the fp8 splits (set by kernel() from input stats
# before build; folded into eviction rescales on-device).  fp8e4 here is
# IEEE e4m3 with max normal 240, so scale targets keep |vals| <= ~136.
SCALES = {"sx": 16.0, "spkv": 2048.0, "swq": 32768.0, "swp": 4096.0,
          "so": 64.0}

_cache = {}


def build_nc():
    nc = bacc.Bacc(None, target_bir_lowering=False, debug=False)

    comp_a = "a" in COMP
    comp_b = "b" in COMP
    comp_e = "e" in COMP

    # eviction rescales
    cA = 1.0 / (SCALES["sx"] * SCALES["spkv"]) if comp_a else 1.0
    cB = 1.0 / (SCALES["sx"] * SCALES["swq"]) if comp_b else 1.0
    cE = 1.0 / (SCALES["so"] * SCALES["swp"]) if comp_e else 1.0
    inv_so = 1.0 / SCALES["so"] if comp_e else 1.0

    if comp_a:
        # x rows: [hi(1024)|lo(1024)]; projkv rows: [lo(512)|hi(512)]
        x_d = nc.dram_tensor("x", [N, 2 * D], F8, kind="ExternalInput")
        pkv_d = nc.dram_tensor("projkv", [N, 4 * K], F8, kind="ExternalInput")
    else:
        x_d = nc.dram_tensor("x", [N, D], MMDT, kind="ExternalInput")
        pkv_d = nc.dram_tensor("projkv", [N, 2 * K], MMDT, kind="ExternalInput")
    if comp_b:
        # xt8 rows: per nb-block [lo(512)|hi(512)]; wq8 rows: [hi|lo]
        xt_d = nc.dram_tensor("xt", [D, 2 * N], F8, kind="ExternalInput")
        wq_d = nc.dram_tensor("wq", [D, 2 * DG], F8, kind="ExternalInput")
    else:
        xt_d = nc.dram_tensor("xt", [D, N], MMDT, kind="ExternalInput")
        wq_d = nc.dram_tensor("wq", [D, DG], MMDT, kind="ExternalInput")
    wk_d = nc.dram_tensor("wk", [D, DG], MMDT, kind="ExternalInput")
    wv_d = nc.dram_tensor("wv", [D, DG], MMDT, kind="ExternalInput")
    wp_d = nc.dram_tensor("wproj", [DG, 2 * D] if comp_e else [DG, D],
                          F8 if comp_e else MMDT, kind="ExternalInput")
    out_d = nc.dram_tensor("out", [N, D], MMDT, kind="ExternalOutput")

    XW = 2 * D if comp_a else D          # x tile cols per chunk
    KVW = 4 * K if comp_a else 2 * K     # kv tile cols per chunk
    XTW = 2 * DG if comp_b else DG       # xt cols per dd
    WQW = 2 * DG if comp_b else DG
    WPW = 2 * D if comp_e else D
    OTW = 2 * DG if comp_e else DG

    with tile.TileContext(nc) as tc:
        from contextlib import ExitStack
        with ExitStack() as ctx:
            res = ctx.enter_context(tc.tile_pool(name="res", bufs=1))
            ones_sb = res.tile([P, 1], MMDT, tag="ones")
            nc.vector.memset(ones_sb[:], inv_so)
            from concourse.masks import make_identity
            id_mm = res.tile([P, P], MMDT, tag="id_mm")
            make_identity(nc, id_mm[:])

            wq_sb = res.tile([P, 8 * WQW], F8 if comp_b else MMDT, tag="wq")
            wk_sb = res.tile([P, 8 * DG], MMDT, tag="wk")
            wv_sb = res.tile([P, 8 * DG], MMDT, tag="wv")
            wproj_sb = res.tile([P, 4 * WPW], F8 if comp_e else MMDT,
                                tag="wproj")
            kprojT_sb = res.tile([P, 4 * K], MMDT, tag="kprojT")
            vproj_sb = res.tile([P, 2 * DG], MMDT, tag="vproj")
            xcxv_sb = res.tile([P, 8 * 2 * K], MMDT, tag="xcxv")

            wqg = (wq_sb[:].rearrange("p (dd hl j) -> p dd hl j", dd=8, hl=2)
                   if comp_b else None)
            wpg = (wproj_sb[:].rearrange("p (c hl j) -> p c hl j", c=4, hl=2)
                   if comp_e else None)

            qtp = ctx.enter_context(tc.tile_pool(name="qtp", bufs=8))
            op_ = ctx.enter_context(tc.tile_pool(name="op", bufs=8))
            otbp = (ctx.enter_context(tc.tile_pool(name="otb", bufs=4))
                    if comp_e else None)
            otp = ctx.enter_context(tc.tile_pool(name="otp", bufs=8))
            outp = ctx.enter_context(tc.tile_pool(name="outp", bufs=3))
            rcp = ctx.enter_context(tc.tile_pool(name="rcp", bufs=2))

            def load_w(dst, src, nchunk, w):
                # dst[p, c*w + j] = src[c*128 + p, j]
                nc.sync.dma_start(
                    out=dst[:].rearrange("p (c j) -> p c j", c=nchunk),
                    in_=src[:, :].rearrange("(c p) j -> p c j", p=P))

            # pexp/scp pools are created mid-build (inside the phase-A
            # context, once the pa banks are free) so scores(0) can
            # interleave with A2; sc_block resolves them via loop_pools
            loop_pools = {}
            scp_ctx = ExitStack()

            def sc_block(qt, h, pexps):
                pexp_p, scp = loop_pools["pexp"], loop_pools["scp"]
                jc, p0 = h // 2, (h % 2) * DH
                for fc in range(2):
                    st = scp.tile([P, DG], F32, tag="sc")
                    nc.tensor.matmul(
                        st[:],
                        lhsT=kprojT_sb[p0:p0 + DH,
                                       jc * K + fc * P: jc * K + (fc + 1) * P],
                        rhs=qt[p0:p0 + DH, jc * DG:(jc + 1) * DG],
                        start=True, stop=True)
                    pexp = pexp_p.tile([P, DG], MMDT, tag=f"px{h}_{fc}")
                    nc.scalar.activation(pexp[:], st[:], Exp)
                    pexps[(h, fc)] = pexp

            def b_block(xt, qt, jc, acc_pool, dd_range=(0, 8), accq=None,
                        on_act=False):
                if accq is None:
                    accq = acc_pool.tile([P, DG], F32, tag="acc")
                d0, d1 = dd_range
                js = slice(jc * P, (jc + 1) * P)
                if comp_b:
                    xtg = xt[:].rearrange("p (dd hl j) -> p dd hl j",
                                          dd=8, hl=2)
                    seq = []
                    for dd in range(d0, d1):
                        seq.append((wqg[:, dd, :, js], xtg[:, dd, :, :]))
                        if dd % 2 == 1:
                            seq.append((wqg[:, dd - 1:dd + 1, 0, js],
                                        xtg[:, dd - 1:dd + 1, 1, :]))
                    for i, (lhsT, rhs) in enumerate(seq):
                        nc.tensor.matmul(
                            accq[:], lhsT=lhsT, rhs=rhs,
                            start=(i == 0 and d0 == 0),
                            stop=(i == len(seq) - 1 and d1 == 8),
                            perf_mode=DR)
                else:
                    for dd in range(d0, d1):
                        nc.tensor.matmul(
                            accq[:],
                            lhsT=wq_sb[:, dd * DG + jc * P: dd * DG + (jc + 1) * P],
                            rhs=xt[:, dd * DG:(dd + 1) * DG],
                            start=(dd == 0), stop=(dd == 7))
                if d1 == 8:
                    dst = qt[:, jc * DG:(jc + 1) * DG]
                    if on_act:
                        nc.scalar.mul(dst, accq[:], cB)
                    elif comp_b:
                        nc.vector.tensor_scalar_mul(dst, accq[:], cB)
                    else:
                        nc.vector.tensor_copy(dst, accq[:])
                return accq

            # ============ Phase B (all n-blocks) + A-tile prefetch ========
            xin_ctx = ExitStack()
            xin = xin_ctx.enter_context(tc.tile_pool(name="xin", bufs=3))
            xkv = {}

            def load_xkv(b4):
                x4 = xin.tile([P, 4 * XW], F8 if comp_a else MMDT,
                              tag="x4", name=f"x4_{b4}")
                kv4 = xin.tile([P, 4 * KVW], F8 if comp_a else MMDT,
                               tag="kv4", name=f"kv4_{b4}")
                nc.sync.dma_start(
                    out=x4[:].rearrange("p (c j) -> p c j", c=4),
                    in_=x_d[b4 * 512:(b4 + 1) * 512, :]
                        .rearrange("(c p) j -> p c j", p=P))
                nc.scalar.dma_start(
                    out=kv4[:].rearrange("p (c j) -> p c j", c=4),
                    in_=pkv_d[b4 * 512:(b4 + 1) * 512, :]
                        .rearrange("(c p) j -> p c j", p=P))
                xkv[b4] = (x4, kv4)

            qts = {}
            with ExitStack() as bctx:
                xtp = bctx.enter_context(tc.tile_pool(name="xtp", bufs=4))
                bpp_ctx = ExitStack()
                bpp = bpp_ctx.enter_context(
                    tc.tile_pool(name="bpp", bufs=5, space="PSUM"))

                def load_xt(nb, split=False):
                    xt = xtp.tile([P, 8 * XTW], F8 if comp_b else MMDT,
                                  tag="xt", name=f"xt{nb}")
                    nhalf = 2 if split else 1
                    for hf in range(nhalf):
                        dph = 8 // nhalf
                        nc.sync.dma_start(
                            out=xt[:, hf * dph * XTW:(hf + 1) * dph * XTW]
                                .rearrange("p (d j) -> p d j", d=dph),
                            in_=xt_d[hf * dph * P:(hf + 1) * dph * P,
                                     nb * XTW:(nb + 1) * XTW]
                                .rearrange("(d p) j -> p d j", p=P))
                    return xt

                # PE p-state warm-up while wq/xt0 stream in
                warm = bpp.tile([P, P], F32, tag="warm", bufs=1)
                for _ in range(12):
                    nc.tensor.matmul(warm[:1, :P], lhsT=ones_sb[:],
                                     rhs=id_mm[:], start=True, stop=True)
                # wq/xt0 stream in dd-pair quarters, wq on the Act queue and
                # xt0 on SP so issue delays overlap; B(0) runs in matching
                # dd-pair stages right behind the quarters
                xt0 = xtp.tile([P, 8 * XTW], F8 if comp_b else MMDT,
                               tag="xt", name="xt0")
                xt_tiles = {0: xt0}
                for ei in range(8):
                    # first eighths on SP: the Act queue is blocked by the
                    # ~1.3us LoadActFuncSet at kernel start
                    (nc.sync if ei < 2 else nc.scalar).dma_start(
                        out=wq_sb[:, ei * WQW:(ei + 1) * WQW],
                        in_=wq_d[ei * P:(ei + 1) * P, :])
                    nc.sync.dma_start(
                        out=xt0[:, ei * XTW:(ei + 1) * XTW],
                        in_=xt_d[ei * P:(ei + 1) * P, :XTW])
                xt_tiles.update({nb: load_xt(nb) for nb in range(1, 4)})
                for nb in range(NB):
                    if nb + 4 < NB:
                        xt_tiles[nb + 4] = load_xt(nb + 4)
                    if nb in (2, 3, 4):
                        load_xkv(nb - 2)      # prefetch A tiles b4 0..2
                    qts[nb] = qtp.tile([P, 4 * DG], MMDT, tag="qt",
                                       name=f"qt{nb}")
                    if nb == 0:
                        # eight single-dd stages chasing the eighth loads
                        accqs = [b_block(xt_tiles[0], qts[0], jc, bpp,
                                         dd_range=(0, 1)) for jc in range(4)]
                        for dd in range(1, 8):
                            for jc in range(4):
                                b_block(xt_tiles[0], qts[0], jc, bpp,
                                        dd_range=(dd, dd + 1),
                                        accq=accqs[jc])
                    else:
                        for jc in range(4):
                            b_block(xt_tiles[nb], qts[nb], jc, bpp,
                                    on_act=(nb == 7 and jc % 2 == 1))
                    del xt_tiles[nb]
                bpp_ctx.close()

            # ---------------- Phase A ----------------
            with ExitStack() as actx:
                pa_ctx = ExitStack()
                pa = pa_ctx.enter_context(tc.tile_pool(name="pa", bufs=1, space="PSUM"))
                accs = [pa.tile([P, 2 * K], F32, tag=f"pa{dd}", name=f"pa{dd}")
                        for dd in range(8)]

                def a_corr(xg, kvg, c, dd, start):
                    ds = slice(dd * P, (dd + 1) * P)
                    nc.tensor.matmul(
                        accs[dd][:], lhsT=xg[:, c, :, ds],
                        rhs=kvg[:, c, :, :],
                        start=start, stop=False, perf_mode=DR)

                def a_main(xg, kvg, ca, dd, stop):
                    ds = slice(dd * P, (dd + 1) * P)
                    nc.tensor.matmul(
                        accs[dd][:], lhsT=xg[:, ca:ca + 2, 0, ds],
                        rhs=kvg[:, ca:ca + 2, 1, :],
                        start=False, stop=stop, perf_mode=DR)

                def a_bf16(x4, kv4, c, dd, start, stop):
                    nc.tensor.matmul(
                        accs[dd][:],
                        lhsT=x4[:, c * D + dd * P: c * D + (dd + 1) * P],
                        rhs=kv4[:, c * 2 * K:(c + 1) * 2 * K],
                        start=start, stop=stop)

                for b4 in range(8):
                    if b4 not in xkv:
                        load_xkv(b4)
                    x4, kv4 = xkv.pop(b4)
                    if b4 == 4:
                        # weight loads for A2/E slot into A's DMA slack
                        load_w(wk_sb, wk_d, 8, DG)
                        load_w(wv_sb, wv_d, 8, DG)
                        load_w(wproj_sb, wp_d, 4, WPW)
                    if comp_a:
                        xg = x4[:].rearrange("p (c hl j) -> p c hl j",
                                             c=4, hl=2)
                        kvg = kv4[:].rearrange("p (c hl j) -> p c hl j",
                                               c=4, hl=2)
                    if b4 == 7:
                        # dd-major on the last batch: acc[dd] stops after its
                        # 4 chunks, so evictions overlap the remaining matmuls
                        for dd in range(8):
                            if comp_a:
                                for c in range(4):
                                    a_corr(xg, kvg, c, dd, start=False)
                                    if c % 2:
                                        a_main(xg, kvg, c - 1, dd,
                                               stop=(c == 3))
                            else:
                                for c in range(4):
                                    a_bf16(x4, kv4, c, dd, start=False,
                                           stop=(c == 3))
                            dst = xcxv_sb[:, dd * 2 * K:(dd + 1) * 2 * K]
                            if dd % 2:
                                if cA != 1.0:
                                    nc.vector.tensor_scalar_mul(
                                        dst, accs[dd][:], cA)
                                else:
                                    nc.vector.tensor_copy(dst, accs[dd][:])
                            else:
                                if cA != 1.0:
                                    nc.scalar.mul(dst, accs[dd][:], cA)
                                else:
                                    nc.scalar.copy(out=dst, in_=accs[dd][:])
                    else:
                        for c in range(4):
                            first = (b4 == 0 and c == 0)
                            for dd in range(8):
                                if comp_a:
                                    a_corr(xg, kvg, c, dd, start=first)
                                else:
                                    a_bf16(x4, kv4, c, dd, start=first,
                                           stop=False)
                            if comp_a and c % 2:
                                for dd in range(8):
                                    a_main(xg, kvg, c - 1, dd, stop=False)
                xin_ctx.close()
                # Phase A2 — release the A accumulators' banks first.
                # scores(0) interleave with the A2 groups: each jc's kprojT
                # eviction immediately feeds that jc's two score blocks, so
                # the Act exp queue starts draining ~7us before the loop and
                # the loop ramp isn't exp-bound.
                pa_ctx.close()
                loop_pools["pexp"] = ctx.enter_context(
                    tc.tile_pool(name="pexp", bufs=2))
                accp = ctx.enter_context(tc.tile_pool(name="accp", bufs=3,
                                                      space="PSUM", side="right"))
                pop = ctx.enter_context(tc.tile_pool(name="pop", bufs=2,
                                                     space="PSUM"))
                smp = ctx.enter_context(tc.tile_pool(name="smp", bufs=1,
                                                     space="PSUM"))
                # scp last on the left stack: released after the final score
                # block to make room for the epilogue transpose pool
                loop_pools["scp"] = scp_ctx.enter_context(
                    tc.tile_pool(name="scp", bufs=2, space="PSUM"))
                pexps0 = {}
                # A2 accumulates into the loop's po-tag tiles (no extra
                # PSUM pool needed alongside the loop pools)
                for jc in range(4):
                    acc = pop.tile([P, DG], F32, tag="po", name=f"kpj{jc}")
                    for dd in range(8):
                        nc.tensor.matmul(
                            acc[:, :K],
                            lhsT=wk_sb[:, dd * DG + jc * P: dd * DG + (jc + 1) * P],
                            rhs=xcxv_sb[:, dd * 2 * K: dd * 2 * K + K],
                            start=(dd == 0), stop=(dd == 7))
                    if jc % 2:
                        nc.scalar.copy(out=kprojT_sb[:, jc * K:(jc + 1) * K],
                                       in_=acc[:, :K])
                    else:
                        nc.vector.tensor_copy(kprojT_sb[:, jc * K:(jc + 1) * K],
                                              acc[:, :K])
                    sc_block(qts[0], 2 * jc, pexps0)
                    sc_block(qts[0], 2 * jc + 1, pexps0)
                for fc in range(2):
                    acc2 = pop.tile([P, DG], F32, tag="po", name=f"vpj{fc}")
                    for dd in range(8):
                        nc.tensor.matmul(
                            acc2[:],
                            lhsT=xcxv_sb[:, dd * 2 * K + K + fc * P:
                                         dd * 2 * K + K + (fc + 1) * P],
                            rhs=wv_sb[:, dd * DG:(dd + 1) * DG],
                            start=(dd == 0), stop=(dd == 7))
                    if fc:
                        nc.scalar.copy(out=vproj_sb[:, fc * DG:(fc + 1) * DG],
                                       in_=acc2[:])
                    else:
                        nc.vector.tensor_copy(vproj_sb[:, fc * DG:(fc + 1) * DG],
                                              acc2[:])

            def split_ot(src, nb, nn2, on_pool=True):
                # src: bf16 [128, DG] (so-scaled oT); write [hi|lo] per
                # 128-chunk into an OTW fp8 tile.  GPSIMD when src is SBUF
                # (keeps DVE/Act under the loop's PE time), Act+DVE for the
                # PSUM-sourced epilogue tiles.
                ot = otp.tile([P, OTW], F8, tag="ot", name=f"ot{nb}_{nn2}")
                ot4 = ot[:].rearrange("p (c hl j) -> p c hl j", c=4, hl=2)
                src3 = src[:].rearrange("p (c j) -> p c j", c=4)
                if on_pool:
                    nc.gpsimd.tensor_copy(ot4[:, :, 0, :], src3)
                    nc.gpsimd.tensor_tensor(
                        out=ot4[:, :, 1, :], in0=src3, in1=ot4[:, :, 0, :],
                        op=mybir.AluOpType.subtract)
                else:
                    nc.scalar.copy(out=ot4[:, :, 0, :], in_=src3)
                    nc.vector.tensor_tensor(
                        out=ot4[:, :, 1, :], in0=src3, in1=ot4[:, :, 0, :],
                        op=mybir.AluOpType.subtract)
                return ot

            def d_group(nb, nn2, pexps, sp, recips, skip_t=False):
                po = pop.tile([P, DG], F32, tag="po")
                for h in range(HL):
                    for fc in range(2):
                        px = pexps[(h, fc)]
                        nc.tensor.matmul(
                            po[:, h * DH:(h + 1) * DH],
                            lhsT=px[:, nn2 * P:(nn2 + 1) * P],
                            rhs=vproj_sb[:, fc * DG + h * DH:
                                         fc * DG + (h + 1) * DH],
                            start=(fc == 0), stop=(fc == 1))
                        nc.tensor.matmul(
                            sp[:, nn2 * HL + h: nn2 * HL + h + 1],
                            lhsT=px[:, nn2 * P:(nn2 + 1) * P],
                            rhs=ones_sb[:],
                            start=(fc == 0), stop=(fc == 1))
                nc.vector.reciprocal(
                    recips[:, nn2 * HL:(nn2 + 1) * HL],
                    sp[:, nn2 * HL:(nn2 + 1) * HL])
                o_t = op_.tile([P, DG], MMDT, tag="o", name=f"o{nb}_{nn2}")
                nc.vector.tensor_tensor(
                    out=o_t[:].rearrange("p (h j) -> p h j", h=HL),
                    in0=po[:].rearrange("p (h j) -> p h j", h=HL),
                    in1=recips[:, nn2 * HL:(nn2 + 1) * HL]
                        .broadcast_to([P, HL, DH]),
                    op=mybir.AluOpType.mult)
                if skip_t:
                    return o_t
                if comp_e:
                    otb = otbp.tile([P, DG], MMDT, tag="otb")
                    nc.sync.dma_start_transpose(
                        out=otb[:].rearrange("p (c j) -> p c j", c=4),
                        in_=o_t[:])
                    return split_ot(otb, nb, nn2)
                ot = otp.tile([P, DG], MMDT, tag="ot", name=f"ot{nb}_{nn2}")
                nc.sync.dma_start_transpose(
                    out=ot[:].rearrange("p (c j) -> p c j", c=4),
                    in_=o_t[:])
                return ot

            def e_group(nb, nn2, ot, last=False, store_eng=None):
                ci = nb * 4 + nn2
                outsb = outp.tile([P, D], MMDT, tag="outsb")
                otg = (ot[:].rearrange("p (c hl j) -> p c hl j", c=4, hl=2)
                       if comp_e else None)
                for half in range(2):
                    hs = slice(half * DG, (half + 1) * DG)
                    pe_acc = accp.tile([P, DG], F32, tag="acc")
                    if comp_e:
                        i, n_i = 0, 6
                        for c in range(0, 4, 2):
                            for lhsT, rhs in (
                                    (otg[:, c, :, :], wpg[:, c, :, hs]),
                                    (otg[:, c + 1, :, :], wpg[:, c + 1, :, hs]),
                                    (otg[:, c:c + 2, 0, :],
                                     wpg[:, c:c + 2, 1, hs])):
                                nc.tensor.matmul(
                                    pe_acc[:], lhsT=lhsT, rhs=rhs,
                                    start=(i == 0), stop=(i == n_i - 1),
                                    perf_mode=DR)
                                i += 1
                    else:
                        for jc2 in range(4):
                            nc.tensor.matmul(
                                pe_acc[:],
                                lhsT=ot[:, jc2 * P:(jc2 + 1) * P],
                                rhs=wproj_sb[:, jc2 * D + half * DG:
                                             jc2 * D + (half + 1) * DG],
                                start=(jc2 == 0), stop=(jc2 == 3))
                    if last:
                        # fast tail: evict on both engines, store each half as
                        # soon as it lands (HWDGE has lower fixed latency)
                        if half == 0:
                            if cE != 1.0:
                                nc.scalar.mul(outsb[:, :DG], pe_acc[:], cE)
                            else:
                                nc.scalar.copy(out=outsb[:, :DG], in_=pe_acc[:])
                        else:
                            if cE != 1.0:
                                nc.vector.tensor_scalar_mul(
                                    outsb[:, DG:], pe_acc[:], cE)
                            else:
                                nc.vector.tensor_copy(outsb[:, DG:], pe_acc[:])
                        nc.sync.dma_start(
                            out=out_d[ci * P:(ci + 1) * P,
                                      half * DG:(half + 1) * DG],
                            in_=outsb[:, half * DG:(half + 1) * DG])
                    else:
                        if cE != 1.0:
                            nc.vector.tensor_scalar_mul(
                                outsb[:, hs], pe_acc[:], cE)
                        else:
                            nc.vector.tensor_copy(outsb[:, hs], pe_acc[:])
                if not last:
                    (store_eng or nc.sync).dma_start(
                        out=out_d[ci * P:(ci + 1) * P, :], in_=outsb[:])

            # ---------------- merged loop (nb = 0..6) ----------------
            # scores run one n-block ahead of D/E: D(nb) consumes exps that
            # had a full iteration of Act time to drain
            prev_ots = None
            all_pexps = {0: pexps0}
            for nb in range(NB - 1):
                pexps = all_pexps.pop(nb)
                all_pexps[nb + 1] = {}
                cur_ots = []
                sp = smp.tile([P, 4 * HL], F32, tag="sums")
                recips = rcp.tile([P, 4 * HL], F32, tag="recips")
                for h in range(HL):
                    sc_block(qts[nb + 1], h, all_pexps[nb + 1])
                if prev_ots is not None:
                    for nn2 in range(4):
                        e_group(nb - 1, nn2, prev_ots[nn2])
                for nn2 in range(4):
                    cur_ots.append(d_group(nb, nn2, pexps, sp, recips))
                prev_ots = cur_ots
            pexps7 = all_pexps.pop(NB - 1)
            # ---- last iter: E(6,3) placed after D(7) to cover latency;
            # o-transposes for block 7 run on the PE (via the freed score
            # banks) instead of the ~3us-latency DMA-transpose path
            scp_ctx.close()
            trp = ctx.enter_context(tc.tile_pool(name="trp", bufs=2,
                                                 space="PSUM"))
            sp = smp.tile([P, 4 * HL], F32, tag="sums")
            recips = rcp.tile([P, 4 * HL], F32, tag="recips")
            for nn2 in range(3):
                e_group(NB - 2, nn2, prev_ots[nn2])
            o7 = [d_group(NB - 1, nn2, pexps7, sp, recips, skip_t=True)
                  for nn2 in range(4)]
            ots7 = []

            def tr_group(nn2):
                tr = trp.tile([P, DG], MMDT, tag="tr")
                for c in range(4):
                    nc.tensor.transpose(tr[:, c * P:(c + 1) * P],
                                        o7[nn2][:, c * P:(c + 1) * P],
                                        id_mm[:])
                if comp_e:
                    ots7.append(split_ot(tr, NB - 1, nn2, on_pool=False))
                else:
                    ot = otp.tile([P, DG], MMDT, tag="ot", name=f"otz{nn2}")
                    nc.scalar.copy(out=ot[:], in_=tr[:])
                    ots7.append(ot)

            tr_group(0)
            tr_group(1)
            # E(6,3) here: covers the split(0) Act/DVE latency with PE work
            e_group(NB - 2, 3, prev_ots[3], store_eng=nc.scalar)
            tr_group(2)
            tr_group(3)
            for nn2 in range(4):
                e_group(NB - 1, nn2, ots7[nn2], last=True)
    nc.compile()
    return nc


def _np_mm(a):
    return np.ascontiguousarray(np.asarray(a), dtype=mybir.dt.np(MMDT))


def _split8(a, s):
    """a*s split into (hi, lo) fp8 arrays (f32 math, e4m3 rounding)."""
    f8 = mybir.dt.np(F8)
    hi = np.asarray(np.asarray(a, np.float32) * np.float32(s), dtype=f8)
    lo = np.asarray(np.asarray(a, np.float32) * np.float32(s)
                    - hi.astype(np.float32), dtype=f8)
    return hi, lo


def _pow2(target, amax):
    return float(2.0 ** np.round(np.log2(target / float(amax))))


def kernel(x, Wq, Wkv, Wproj, bproj, proj_k, proj_v):
    x = np.asarray(x)
    Wq, Wkv, Wproj = np.asarray(Wq), np.asarray(Wkv), np.asarray(Wproj)
    bproj, proj_k, proj_v = np.asarray(bproj), np.asarray(proj_k), np.asarray(proj_v)

    scale = np.float32(DH ** -0.5)
    comp_a, comp_b, comp_e = ("a" in COMP), ("b" in COMP), ("e" in COMP)

    if "nc" not in _cache:
        # pick power-of-2 scales from input stats before building
        SCALES["sx"] = _pow2(96.0, np.abs(x).max())
        SCALES["spkv"] = _pow2(96.0, max(np.abs(proj_k).max(),
                                         np.abs(proj_v).max()))
        SCALES["swq"] = _pow2(96.0, float(scale) * np.abs(Wq).max())
        SCALES["swp"] = _pow2(96.0, np.abs(Wproj).max())
        SCALES["so"] = 64.0
        _cache["nc"] = build_nc()
    nc = _cache["nc"]

    projkv = np.concatenate([proj_k, proj_v], axis=1)
    if comp_a:
        xhi, xlo = _split8(x, SCALES["sx"])            # [4,4096,1024]
        x8 = np.concatenate([xhi, xlo], axis=2)        # rows [hi|lo]
        phi, plo = _split8(projkv, SCALES["spkv"])     # [4096, 512]
        pkv8 = np.ascontiguousarray(np.concatenate([plo, phi], axis=1))
    if comp_b:
        xt = np.swapaxes(x, 1, 2)                      # [4, 1024, 4096]
        thi, tlo = _split8(xt, SCALES["sx"])
        # per nb-block of 512: [lo|hi]
        t8 = np.stack([tlo.reshape(4, D, NB, DG),
                       thi.reshape(4, D, NB, DG)],
                      axis=3).reshape(4, D, 2 * N)

    in_maps = []
    for c in range(8):
        b, g = c // 2, c % 2
        cols = slice(g * DG, (g + 1) * DG)
        m = {"wk": _np_mm(Wkv[:, :D][:, cols]),
             "wv": _np_mm(Wkv[:, D:][:, cols])}
        if comp_a:
            m["x"] = np.ascontiguousarray(x8[b])
            m["projkv"] = pkv8
        else:
            m["x"] = _np_mm(x[b])
            m["projkv"] = _np_mm(projkv)
        if comp_b:
            m["xt"] = np.ascontiguousarray(t8[b])
            qhi, qlo = _split8(scale * Wq[:, cols], SCALES["swq"])
            m["wq"] = np.ascontiguousarray(
                np.concatenate([qhi, qlo], axis=1))    # rows [hi|lo]
        else:
            m["xt"] = np.ascontiguousarray(_np_mm(x[b]).T)
            m["wq"] = _np_mm(scale * Wq[:, cols])
        if comp_e:
            whi, wlo = _split8(Wproj[cols, :], SCALES["swp"])
            m["wproj"] = np.ascontiguousarray(
                np.concatenate([wlo, whi], axis=1))    # rows [lo|hi]
        else:
            m["wproj"] = _np_mm(Wproj[cols, :])
        in_maps.append(m)
    res = run_bass_kernel_spmd(nc, in_maps, list(range(8)),
                               trace=bool(os.environ.get("LINF_TRACE")))
    _cache["last_result"] = res
    outs = [np.asarray(r["out"], dtype=np.float32) for r in res.results]
    full = np.stack([outs[2 * b] + outs[2 * b + 1] for b in range(4)])
    full = full + np.asarray(bproj, np.float32)
    return full.astype(np.float32)


# revision 49
# speedup vs baseline: 1.0065x; 1.0065x over previous
"""Linformer self-attention on 8 Trainium2 NeuronCores.

Problem (hardcoded shapes): x [4,4096,1024] f32; per batch:
  q = scale*(x@Wq); kv = x@Wkv; keys/values compressed 4096->256 via
  proj_k/proj_v; 16-head attention (dh=64, k=256); out @ Wproj + bproj.

Sharding: 8 cores = 4 batches x 2 head-groups (8 heads / 512 cols each).
Each core computes a partial [4096,1024] output (Wproj row-split); host
sums the pair and adds bias.

Numerics: phases A (x.T@projkv), B (Wq.T@xT) and E (oT.T@Wproj) run as
error-compensated fp8e4 DoubleRow matmuls: each operand X is split into
X_hi = fp8(X*s) and X_lo = fp8(X*s - X_hi) (power-of-2 s, exact to
undo).  X@W = sum_c [Xhi_c@Whi_c] + [Xhi_c@Wlo_c + Xlo_c@Whi_c],
dropping the O(eps^2) Xlo@Wlo term; DoubleRow packs 2 row-groups per
instruction at 0.5 cycles/row, so a K-128 chain costs 0.75x bf16 with
~13-bit-mantissa accuracy (better than bf16).  Per 128-row chunk the
lhsT-side tensor is stored [hi|lo] and the rhs-side [lo|hi], so the
main DR (hi_c,hi_c+1) and the correction DR ((hi_c,lo_c)x(lo_c,hi_c))
both slice out of one grid AP.  C, D, A2 and softmax stay bf16.

Phase order is B-first: the whole qT [512,4096] is computed before
phase A, because phase A is DMA-bound (x hi|lo 8MB + projkv 4MB vs
41us of PE) while phase B has DMA slack — its xt tiles stream at
1MB/5.1us of PE.  Phase A's x/kv tiles and the wk/wv/wproj weights
prefetch behind phase B / phase A, so the A stream and the merged
loop never starve.  The merged loop is then scores/E/D only, with
scores running one n-block ahead (D(nb) consumes exps that had a full
iteration of Act time to drain) and scores(0) interleaved into A2;
the oT hi/lo split runs on GPSIMD so DVE (o-normalize + out-evict)
stays under the PE per-iteration time.

Per-core dataflow (all matmuls use out = lhsT.T @ rhs, K<=128 partitions):
  B : qT[512,4096] = Wq_g.T @ xT                   (comp-fp8 DR)
  A : xcxvT[1024,512] = x.T @ [proj_k|proj_v]      (comp-fp8 DR)
  A2: kprojT[512,256] = Wk_g.T @ xcT ; vproj[256,512] = xvT.T @ Wv_g
  C : per (head,fc k-chunk): scoresT[128,512] -> exp (Act) -> pexp bf16
  S : per (n-chunk, head): sums[n,1] = pexp.T @ (1/so)  (N=1 matmuls)
  D : po[n, 8*64] = pexp.T @ vproj_h per head; normalize via DVE
      tensor_tensor with per-head recip broadcast -> o bf16 (so-scaled)
  T : oT via one batched DMA transpose per [128,512] tile; split into
      (hi,lo) fp8 on Act+DVE  (PE transposes for the final block)
  E : out[n,1024] = oT.T-chunks @ Wproj_g           (comp-fp8 DR)
"""

import os
import numpy as np

import concourse.bass as bass
import concourse.mybir as mybir
import concourse.tile as tile
from concourse import bacc
from concourse.bass_utils import run_bass_kernel_spmd

P = 128
N, D, K, DG, DH = 4096, 1024, 256, 512, 64
NB = 8                    # n-blocks of 512
HL = 8                    # heads per core
F32 = mybir.dt.float32

MMDT_NAME = os.environ.get("LINF_MMDT", "bfloat16")
MMDT = getattr(mybir.dt, MMDT_NAME)
Exp = mybir.ActivationFunctionType.Exp
F8 = mybir.dt.float8e4
DR = mybir.MatmulPerfMode.DoubleRow

# which phases run compensated-fp8 DoubleRow (subset of "abe")
COMP = os.environ.get("LINF_COMP", "abe")

# power-of-2 scales for the fp8 splits (set by kernel() from input stats
# before build; folded into eviction rescales on-device).  fp8e4 here is
# IEEE e4m3 with max normal 240, so scale targets keep |vals| <= ~136.
SCALES = {"sx": 16.0, "spkv": 2048.0, "swq": 32768.0, "swp": 4096.0,
          "so": 64.0}

_cache = {}


def build_nc():
    nc = bacc.Bacc(None, target_bir_lowering=False, debug=False)

    comp_a = "a" in COMP
    comp_b = "b" in COMP
    comp_e = "e" in COMP

    # eviction rescales
    cA = 1.0 / (SCALES["sx"] * SCALES["spkv"]) if comp_a else 1.0
    cB = 1.0 / (SCALES["sx"] * SCALES["swq"]) if comp_b else 1.0
    cE = 1.0 / (SCALES["so"] * SCALES["swp"]) if comp_e else 1.0
    inv_so = 1.0 / SCALES["so"] if comp_e else 1.0

    if comp_a:
        # x rows: [hi(1024)|lo(1024)]; projkv rows: [lo(512)|hi(512)]
        x_d = nc.dram_tensor("x", [N, 2 * D], F8, kind="ExternalInput")
        pkv_d = nc.dram_tensor("projkv", [N, 4 * K], F8, kind="ExternalInput")
    else:
        x_d = nc.dram_tensor("x", [N, D], MMDT, kind="ExternalInput")
        pkv_d = nc.dram_tensor("projkv", [N, 2 * K], MMDT, kind="ExternalInput")
    if comp_b:
        # xt8 rows: per nb-block [lo(512)|hi(512)]; wq8 rows: [hi|lo]
        xt_d = nc.dram_tensor("xt", [D, 2 * N], F8, kind="ExternalInput")
        wq_d = nc.dram_tensor("wq", [D, 2 * DG], F8, kind="ExternalInput")
    else:
        xt_d = nc.dram_tensor("xt", [D, N], MMDT, kind="ExternalInput")
        wq_d = nc.dram_tensor("wq", [D, DG], MMDT, kind="ExternalInput")
    wk_d = nc.dram_tensor("wk", [D, DG], MMDT, kind="ExternalInput")
    wv_d = nc.dram_tensor("wv", [D, DG], MMDT, kind="ExternalInput")
    wp_d = nc.dram_tensor("wproj", [DG, 2 * D] if comp_e else [DG, D],
                          F8 if comp_e else MMDT, kind="ExternalInput")
    out_d = nc.dram_tensor("out", [N, D], MMDT, kind="ExternalOutput")

    XW = 2 * D if comp_a else D          # x tile cols per chunk
    KVW = 4 * K if comp_a else 2 * K     # kv tile cols per chunk
    XTW = 2 * DG if comp_b else DG       # xt cols per dd
    WQW = 2 * DG if comp_b else DG
    WPW = 2 * D if comp_e else D
    OTW = 2 * DG if comp_e else DG

    with tile.TileContext(nc) as tc:
        from contextlib import ExitStack
        with ExitStack() as ctx:
            res = ctx.enter_context(tc.tile_pool(name="res", bufs=1))
            ones_sb = res.tile([P, 1], MMDT, tag="ones")
            nc.vector.memset(ones_sb[:], inv_so)
            from concourse.masks import make_identity
            id_mm = res.tile([P, P], MMDT, tag="id_mm")
            make_identity(nc, id_mm[:])

            wq_sb = res.tile([P, 8 * WQW], F8 if comp_b else MMDT, tag="wq")
            wk_sb = res.tile([P, 8 * DG], MMDT, tag="wk")
            wv_sb = res.tile([P, 8 * DG], MMDT, tag="wv")
            wproj_sb = res.tile([P, 4 * WPW], F8 if comp_e else MMDT,
                                tag="wproj")
            kprojT_sb = res.tile([P, 4 * K], MMDT, tag="kprojT")
            vproj_sb = res.tile([P, 2 * DG], MMDT, tag="vproj")
            xcxv_sb = res.tile([P, 8 * 2 * K], MMDT, tag="xcxv")

            wqg = (wq_sb[:].rearrange("p (dd hl j) -> p dd hl j", dd=8, hl=2)
                   if comp_b else None)
            wpg = (wproj_sb[:].rearrange("p (c hl j) -> p c hl j", c=4, hl=2)
                   if comp_e else None)

            qtp = ctx.enter_context(tc.tile_pool(name="qtp", bufs=8))
            op_ = ctx.enter_context(tc.tile_pool(name="op", bufs=8))
            otbp = (ctx.enter_context(tc.tile_pool(name="otb", bufs=4))
                    if comp_e else None)
            otp = ctx.enter_context(tc.tile_pool(name="otp", bufs=8))
            outp = ctx.enter_context(tc.tile_pool(name="outp", bufs=3))
            rcp = ctx.enter_context(tc.tile_pool(name="rcp", bufs=2))

            def load_w(dst, src, nchunk, w):
                # dst[p, c*w + j] = src[c*128 + p, j]
                nc.sync.dma_start(
                    out=dst[:].rearrange("p (c j) -> p c j", c=nchunk),
                    in_=src[:, :].rearrange("(c p) j -> p c j", p=P))

            # pexp/scp pools are created mid-build (inside the phase-A
            # context, once the pa banks are free) so scores(0) can
            # interleave with A2; sc_block resolves them via loop_pools
            loop_pools = {}
            scp_ctx = ExitStack()

            def sc_block(qt, h, pexps):
                pexp_p, scp = loop_pools["pexp"], loop_pools["scp"]
                jc, p0 = h // 2, (h % 2) * DH
                for fc in range(2):
                    st = scp.tile([P, DG], F32, tag="sc")
                    nc.tensor.matmul(
                        st[:],
                        lhsT=kprojT_sb[p0:p0 + DH,
                                       jc * K + fc * P: jc * K + (fc + 1) * P],
                        rhs=qt[p0:p0 + DH, jc * DG:(jc + 1) * DG],
                        start=True, stop=True)
                    pexp = pexp_p.tile([P, DG], MMDT, tag=f"px{h}_{fc}")
                    nc.scalar.activation(pexp[:], st[:], Exp)
                    pexps[(h, fc)] = pexp

            def b_block(xt, qt, jc, acc_pool, dd_range=(0, 8), accq=None,
                        on_act=False):
                if accq is None:
                    accq = acc_pool.tile([P, DG], F32, tag="acc")
                d0, d1 = dd_range
                js = slice(jc * P, (jc + 1) * P)
                if comp_b:
                    xtg = xt[:].rearrange("p (dd hl j) -> p dd hl j",
                                          dd=8, hl=2)
                    seq = []
                    for dd in range(d0, d1):
                        seq.append((wqg[:, dd, :, js], xtg[:, dd, :, :]))
                        if dd % 2 == 1:
                            seq.append((wqg[:, dd - 1:dd + 1, 0, js],
                                        xtg[:, dd - 1:dd + 1, 1, :]))
                    for i, (lhsT, rhs) in enumerate(seq):
                        nc.tensor.matmul(
                            accq[:], lhsT=lhsT, rhs=rhs,
                            start=(i == 0 and d0 == 0),
                            stop=(i == len(seq) - 1 and d1 == 8),
                            perf_mode=DR)
                else:
                    for dd in range(d0, d1):
                        nc.tensor.matmul(
                            accq[:],
                            lhsT=wq_sb[:, dd * DG + jc * P: dd * DG + (jc + 1) * P],
                            rhs=xt[:, dd * DG:(dd + 1) * DG],
                            start=(dd == 0), stop=(dd == 7))
                if d1 == 8:
                    dst = qt[:, jc * DG:(jc + 1) * DG]
                    if on_act:
                        nc.scalar.mul(dst, accq[:], cB)
                    elif comp_b:
                        nc.vector.tensor_scalar_mul(dst, accq[:], cB)
                    else:
                        nc.vector.tensor_copy(dst, accq[:])
                return accq

            # ============ Phase B (all n-blocks) + A-tile prefetch ========
            xin_ctx = ExitStack()
            xin = xin_ctx.enter_context(tc.tile_pool(name="xin", bufs=3))
            xkv = {}

            def load_xkv(b4):
                x4 = xin.tile([P, 4 * XW], F8 if comp_a else MMDT,
                              tag="x4", name=f"x4_{b4}")
                kv4 = xin.tile([P, 4 * KVW], F8 if comp_a else MMDT,
                               tag="kv4", name=f"kv4_{b4}")
                nc.sync.dma_start(
                    out=x4[:].rearrange("p (c j) -> p c j", c=4),
                    in_=x_d[b4 * 512:(b4 + 1) * 512, :]
                        .rearrange("(c p) j -> p c j", p=P))
                nc.scalar.dma_start(
                    out=kv4[:].rearrange("p (c j) -> p c j", c=4),
                    in_=pkv_d[b4 * 512:(b4 + 1) * 512, :]
                        .rearrange("(c p) j -> p c j", p=P))
                xkv[b4] = (x4, kv4)

            qts = {}
            with ExitStack() as bctx:
                xtp = bctx.enter_context(tc.tile_pool(name="xtp", bufs=4))
                bpp_ctx = ExitStack()
                bpp = bpp_ctx.enter_context(
                    tc.tile_pool(name="bpp", bufs=5, space="PSUM"))

                def load_xt(nb, split=False):
                    xt = xtp.tile([P, 8 * XTW], F8 if comp_b else MMDT,
                                  tag="xt", name=f"xt{nb}")
                    nhalf = 2 if split else 1
                    for hf in range(nhalf):
                        dph = 8 // nhalf
                        nc.sync.dma_start(
                            out=xt[:, hf * dph * XTW:(hf + 1) * dph * XTW]
                                .rearrange("p (d j) -> p d j", d=dph),
                            in_=xt_d[hf * dph * P:(hf + 1) * dph * P,
                                     nb * XTW:(nb + 1) * XTW]
                                .rearrange("(d p) j -> p d j", p=P))
                    return xt

                # PE p-state warm-up while wq/xt0 stream in
                warm = bpp.tile([P, P], F32, tag="warm", bufs=1)
                for _ in range(12):
                    nc.tensor.matmul(warm[:1, :P], lhsT=ones_sb[:],
                                     rhs=id_mm[:], start=True, stop=True)
                # wq/xt0 stream in dd-pair quarters, wq on the Act queue and
                # xt0 on SP so issue delays overlap; B(0) runs in matching
                # dd-pair stages right behind the quarters
                xt0 = xtp.tile([P, 8 * XTW], F8 if comp_b else MMDT,
                               tag="xt", name="xt0")
                xt_tiles = {0: xt0}
                for ei in range(8):
                    nc.scalar.dma_start(
                        out=wq_sb[:, ei * WQW:(ei + 1) * WQW],
                        in_=wq_d[ei * P:(ei + 1) * P, :])
                    nc.sync.dma_start(
                        out=xt0[:, ei * XTW:(ei + 1) * XTW],
                        in_=xt_d[ei * P:(ei + 1) * P, :XTW])
                xt_tiles.update({nb: load_xt(nb) for nb in range(1, 4)})
                for nb in range(NB):
                    if nb + 4 < NB:
                        xt_tiles[nb + 4] = load_xt(nb + 4)
                    if nb in (2, 3, 4):
                        load_xkv(nb - 2)      # prefetch A tiles b4 0..2
                    qts[nb] = qtp.tile([P, 4 * DG], MMDT, tag="qt",
                                       name=f"qt{nb}")
                    if nb == 0:
                        # eight single-dd stages chasing the eighth loads
                        accqs = [b_block(xt_tiles[0], qts[0], jc, bpp,
                                         dd_range=(0, 1)) for jc in range(4)]
                        for dd in range(1, 8):
                            for jc in range(4):
                                b_block(xt_tiles[0], qts[0], jc, bpp,
                                        dd_range=(dd, dd + 1),
                                        accq=accqs[jc])
                    else:
                        for jc in range(4):
                            b_block(xt_tiles[nb], qts[nb], jc, bpp,
                                    on_act=(nb == 7 and jc % 2 == 1))
                    del xt_tiles[nb]
                bpp_ctx.close()

            # ---------------- Phase A ----------------
            with ExitStack() as actx:
                pa_ctx = ExitStack()
                pa = pa_ctx.enter_context(tc.tile_pool(name="pa", bufs=1, space="PSUM"))
                accs = [pa.tile([P, 2 * K], F32, tag=f"pa{dd}", name=f"pa{dd}")
                        for dd in range(8)]

                def a_corr(xg, kvg, c, dd, start):
                    ds = slice(dd * P, (dd + 1) * P)
                    nc.tensor.matmul(
                        accs[dd][:], lhsT=xg[:, c, :, ds],
                        rhs=kvg[:, c, :, :],
                        start=start, stop=False, perf_mode=DR)

                def a_main(xg, kvg, ca, dd, stop):
                    ds = slice(dd * P, (dd + 1) * P)
                    nc.tensor.matmul(
                        accs[dd][:], lhsT=xg[:, ca:ca + 2, 0, ds],
                        rhs=kvg[:, ca:ca + 2, 1, :],
                        start=False, stop=stop, perf_mode=DR)

                def a_bf16(x4, kv4, c, dd, start, stop):
                    nc.tensor.matmul(
                        accs[dd][:],
                        lhsT=x4[:, c * D + dd * P: c * D + (dd + 1) * P],
                        rhs=kv4[:, c * 2 * K:(c + 1) * 2 * K],
                        start=start, stop=stop)

                for b4 in range(8):
                    if b4 not in xkv:
                        load_xkv(b4)
                    x4, kv4 = xkv.pop(b4)
                    if b4 == 4:
                        # weight loads for A2/E slot into A's DMA slack
                        load_w(wk_sb, wk_d, 8, DG)
                        load_w(wv_sb, wv_d, 8, DG)
                        load_w(wproj_sb, wp_d, 4, WPW)
                    if comp_a:
                        xg = x4[:].rearrange("p (c hl j) -> p c hl j",
                                             c=4, hl=2)
                        kvg = kv4[:].rearrange("p (c hl j) -> p c hl j",
                                               c=4, hl=2)
                    if b4 == 7:
                        # dd-major on the last batch: acc[dd] stops after its
                        # 4 chunks, so evictions overlap the remaining matmuls
                        for dd in range(8):
                            if comp_a:
                                for c in range(4):
                                    a_corr(xg, kvg, c, dd, start=False)
                                    if c % 2:
                                        a_main(xg, kvg, c - 1, dd,
                                               stop=(c == 3))
                            else:
                                for c in range(4):
                                    a_bf16(x4, kv4, c, dd, start=False,
                                           stop=(c == 3))
                            dst = xcxv_sb[:, dd * 2 * K:(dd + 1) * 2 * K]
                            if dd % 2:
                                if cA != 1.0:
                                    nc.vector.tensor_scalar_mul(
                                        dst, accs[dd][:], cA)
                                else:
                                    nc.vector.tensor_copy(dst, accs[dd][:])
                            else:
                                if cA != 1.0:
                                    nc.scalar.mul(dst, accs[dd][:], cA)
                                else:
                                    nc.scalar.copy(out=dst, in_=accs[dd][:])
                    else:
                        for c in range(4):
                            first = (b4 == 0 and c == 0)
                            for dd in range(8):
                                if comp_a:
                                    a_corr(xg, kvg, c, dd, start=first)
                                else:
                                    a_bf16(x4, kv4, c, dd, start=first,
                                           stop=False)
                            if comp_a and c % 2:
                                for dd in range(8):
                                    a_main(xg, kvg, c - 1, dd, stop=False)
                xin_ctx.close()
                # Phase A2 — release the A accumulators' banks first.
                # scores(0) interleave with the A2 groups: each jc's kprojT
                # eviction immediately feeds that jc's two score blocks, so
                # the Act exp queue starts draining ~7us before the loop and
                # the loop ramp isn't exp-bound.
                pa_ctx.close()
                loop_pools["pexp"] = ctx.enter_context(
                    tc.tile_pool(name="pexp", bufs=2))
                accp = ctx.enter_context(tc.tile_pool(name="accp", bufs=3,
                                                      space="PSUM", side="right"))
                pop = ctx.enter_context(tc.tile_pool(name="pop", bufs=2,
                                                     space="PSUM"))
                smp = ctx.enter_context(tc.tile_pool(name="smp", bufs=1,
                                                     space="PSUM"))
                # scp last on the left stack: released after the final score
                # block to make room for the epilogue transpose pool
                loop_pools["scp"] = scp_ctx.enter_context(
                    tc.tile_pool(name="scp", bufs=2, space="PSUM"))
                pexps0 = {}
                # A2 accumulates into the loop's po-tag tiles (no extra
                # PSUM pool needed alongside the loop pools)
                for jc in range(4):
                    acc = pop.tile([P, DG], F32, tag="po", name=f"kpj{jc}")
                    for dd in range(8):
                        nc.tensor.matmul(
                            acc[:, :K],
                            lhsT=wk_sb[:, dd * DG + jc * P: dd * DG + (jc + 1) * P],
                            rhs=xcxv_sb[:, dd * 2 * K: dd * 2 * K + K],
                            start=(dd == 0), stop=(dd == 7))
                    if jc % 2:
                        nc.scalar.copy(out=kprojT_sb[:, jc * K:(jc + 1) * K],
                                       in_=acc[:, :K])
                    else:
                        nc.vector.tensor_copy(kprojT_sb[:, jc * K:(jc + 1) * K],
                                              acc[:, :K])
                    sc_block(qts[0], 2 * jc, pexps0)
                    sc_block(qts[0], 2 * jc + 1, pexps0)
                for fc in range(2):
                    acc2 = pop.tile([P, DG], F32, tag="po", name=f"vpj{fc}")
                    for dd in range(8):
                        nc.tensor.matmul(
                            acc2[:],
                            lhsT=xcxv_sb[:, dd * 2 * K + K + fc * P:
                                         dd * 2 * K + K + (fc + 1) * P],
                            rhs=wv_sb[:, dd * DG:(dd + 1) * DG],
                            start=(dd == 0), stop=(dd == 7))
                    if fc:
                        nc.scalar.copy(out=vproj_sb[:, fc * DG:(fc + 1) * DG],
                                       in_=acc2[:])
                    else:
                        nc.vector.tensor_copy(vproj_sb[:, fc * DG:(fc + 1) * DG],
                                              acc2[:])

            def split_ot(src, nb, nn2, on_pool=True):
                # src: bf16 [128, DG] (so-scaled oT); write [hi|lo] per
                # 128-chunk into an OTW fp8 tile.  GPSIMD when src is SBUF
                # (keeps DVE/Act under the loop's PE time), Act+DVE for the
                # PSUM-sourced epilogue tiles.
                ot = otp.tile([P, OTW], F8, tag="ot", name=f"ot{nb}_{nn2}")
                ot4 = ot[:].rearrange("p (c hl j) -> p c hl j", c=4, hl=2)
                src3 = src[:].rearrange("p (c j) -> p c j", c=4)
                if on_pool:
                    nc.gpsimd.tensor_copy(ot4[:, :, 0, :], src3)
                    nc.gpsimd.tensor_tensor(
                        out=ot4[:, :, 1, :], in0=src3, in1=ot4[:, :, 0, :],
                        op=mybir.AluOpType.subtract)
                else:
                    nc.scalar.copy(out=ot4[:, :, 0, :], in_=src3)
                    nc.vector.tensor_tensor(
                        out=ot4[:, :, 1, :], in0=src3, in1=ot4[:, :, 0, :],
                        op=mybir.AluOpType.subtract)
                return ot

            def d_group(nb, nn2, pexps, sp, recips, skip_t=False):
                po = pop.tile([P, DG], F32, tag="po")
                for h in range(HL):
                    for fc in range(2):
                        px = pexps[(h, fc)]
                        nc.tensor.matmul(
                            po[:, h * DH:(h + 1) * DH],
                            lhsT=px[:, nn2 * P:(nn2 + 1) * P],
                            rhs=vproj_sb[:, fc * DG + h * DH:
                                         fc * DG + (h + 1) * DH],
                            start=(fc == 0), stop=(fc == 1))
                        nc.tensor.matmul(
                            sp[:, nn2 * HL + h: nn2 * HL + h + 1],
                            lhsT=px[:, nn2 * P:(nn2 + 1) * P],
                            rhs=ones_sb[:],
                            start=(fc == 0), stop=(fc == 1))
                nc.vector.reciprocal(
                    recips[:, nn2 * HL:(nn2 + 1) * HL],
                    sp[:, nn2 * HL:(nn2 + 1) * HL])
                o_t = op_.tile([P, DG], MMDT, tag="o", name=f"o{nb}_{nn2}")
                nc.vector.tensor_tensor(
                    out=o_t[:].rearrange("p (h j) -> p h j", h=HL),
                    in0=po[:].rearrange("p (h j) -> p h j", h=HL),
                    in1=recips[:, nn2 * HL:(nn2 + 1) * HL]
                        .broadcast_to([P, HL, DH]),
                    op=mybir.AluOpType.mult)
                if skip_t:
                    return o_t
                if comp_e:
                    otb = otbp.tile([P, DG], MMDT, tag="otb")
                    nc.sync.dma_start_transpose(
                        out=otb[:].rearrange("p (c j) -> p c j", c=4),
                        in_=o_t[:])
                    return split_ot(otb, nb, nn2)
                ot = otp.tile([P, DG], MMDT, tag="ot", name=f"ot{nb}_{nn2}")
                nc.sync.dma_start_transpose(
                    out=ot[:].rearrange("p (c j) -> p c j", c=4),
                    in_=o_t[:])
                return ot

            def e_group(nb, nn2, ot, last=False, store_eng=None):
                ci = nb * 4 + nn2
                outsb = outp.tile([P, D], MMDT, tag="outsb")
                otg = (ot[:].rearrange("p (c hl j) -> p c hl j", c=4, hl=2)
                       if comp_e else None)
                for half in range(2):
                    hs = slice(half * DG, (half + 1) * DG)
                    pe_acc = accp.tile([P, DG], F32, tag="acc")
                    if comp_e:
                        i, n_i = 0, 6
                        for c in range(0, 4, 2):
                            for lhsT, rhs in (
                                    (otg[:, c, :, :], wpg[:, c, :, hs]),
                                    (otg[:, c + 1, :, :], wpg[:, c + 1, :, hs]),
                                    (otg[:, c:c + 2, 0, :],
                                     wpg[:, c:c + 2, 1, hs])):
                                nc.tensor.matmul(
                                    pe_acc[:], lhsT=lhsT, rhs=rhs,
                                    start=(i == 0), stop=(i == n_i - 1),
                                    perf_mode=DR)
                                i += 1
                    else:
                        for jc2 in range(4):
                            nc.tensor.matmul(
                                pe_acc[:],
                                lhsT=ot[:, jc2 * P:(jc2 + 1) * P],
                                rhs=wproj_sb[:, jc2 * D + half * DG:
                                             jc2 * D + (half + 1) * DG],
                                start=(jc2 == 0), stop=(jc2 == 3))
                    if last:
                        # fast tail: evict on both engines, store each half as
                        # soon as it lands (HWDGE has lower fixed latency)
                        if half == 0:
                            if cE != 1.0:
                                nc.scalar.mul(outsb[:, :DG], pe_acc[:], cE)
                            else:
                                nc.scalar.copy(out=outsb[:, :DG], in_=pe_acc[:])
                        else:
                            if cE != 1.0:
                                nc.vector.tensor_scalar_mul(
                                    outsb[:, DG:], pe_acc[:], cE)
                            else:
                                nc.vector.tensor_copy(outsb[:, DG:], pe_acc[:])
                        nc.sync.dma_start(
                            out=out_d[ci * P:(ci + 1) * P,
                                      half * DG:(half + 1) * DG],
                            in_=outsb[:, half * DG:(half + 1) * DG])
                    else:
                        if cE != 1.0:
                            nc.vector.tensor_scalar_mul(
                                outsb[:, hs], pe_acc[:], cE)
                        else:
                            nc.vector.tensor_copy(outsb[:, hs], pe_acc[:])
                if not last:
                    (store_eng or nc.sync).dma_start(
                        out=out_d[ci * P:(ci + 1) * P, :], in_=outsb[:])

            # ---------------- merged loop (nb = 0..6) ----------------
            # scores run one n-block ahead of D/E: D(nb) consumes exps that
            # had a full iteration of Act time to drain
            prev_ots = None
            all_pexps = {0: pexps0}
            for nb in range(NB - 1):
                pexps = all_pexps.pop(nb)
                all_pexps[nb + 1] = {}
                cur_ots = []
                sp = smp.tile([P, 4 * HL], F32, tag="sums")
                recips = rcp.tile([P, 4 * HL], F32, tag="recips")
                for h in range(HL):
                    sc_block(qts[nb + 1], h, all_pexps[nb + 1])
                if prev_ots is not None:
                    for nn2 in range(4):
                        e_group(nb - 1, nn2, prev_ots[nn2])
                for nn2 in range(4):
                    cur_ots.append(d_group(nb, nn2, pexps, sp, recips))
                prev_ots = cur_ots
            pexps7 = all_pexps.pop(NB - 1)
            # ---- last iter: E(6,3) placed after D(7) to cover latency;
            # o-transposes for block 7 run on the PE (via the freed score
            # banks) instead of the ~3us-latency DMA-transpose path
            scp_ctx.close()
            trp = ctx.enter_context(tc.tile_pool(name="trp", bufs=2,
                                                 space="PSUM"))
            sp = smp.tile([P, 4 * HL], F32, tag="sums")
            recips = rcp.tile([P, 4 * HL], F32, tag="recips")
            for nn2 in range(3):
                e_group(NB - 2, nn2, prev_ots[nn2])
            o7 = [d_group(NB - 1, nn2, pexps7, sp, recips, skip_t=True)
                  for nn2 in range(4)]
            ots7 = []

            def tr_group(nn2):
                tr = trp.tile([P, DG], MMDT, tag="tr")
                for c in range(4):
                    nc.tensor.transpose(tr[:, c * P:(c + 1) * P],
                                        o7[nn2][:, c * P:(c + 1) * P],
                                        id_mm[:])
                if comp_e:
                    ots7.append(split_ot(tr, NB - 1, nn2, on_pool=False))
                else:
                    ot = otp.tile([P, DG], MMDT, tag="ot", name=f"otz{nn2}")
                    nc.scalar.copy(out=ot[:], in_=tr[:])
                    ots7.append(ot)

            tr_group(0)
            tr_group(1)
            # E(6,3) here: covers the split(0) Act/DVE latency with PE work
            e_group(NB - 2, 3, prev_ots[3], store_eng=nc.scalar)
            tr_group(2)
            tr_group(3)
            for nn2 in range(4):
                e_group(NB - 1, nn2, ots7[nn2], last=True)
    nc.compile()
    return nc


def _np_mm(a):
    return np.ascontiguousarray(np.asarray(a), dtype=mybir.dt.np(MMDT))


def _split8(a, s):
    """a*s split into (hi, lo) fp8 arrays (f32 math, e4m3 rounding)."""
    f8 = mybir.dt.np(F8)
    hi = np.asarray(np.asarray(a, np.float32) * np.float32(s), dtype=f8)
    lo = np.asarray(np.asarray(a, np.float32) * np.float32(s)
                    - hi.astype(np.float32), dtype=f8)
    return hi, lo


def _pow2(target, amax):
    return float(2.0 ** np.round(np.log2(target / float(amax))))


def kernel(x, Wq, Wkv, Wproj, bproj, proj_k, proj_v):
    x = np.asarray(x)
    Wq, Wkv, Wproj = np.asarray(Wq), np.asarray(Wkv), np.asarray(Wproj)
    bproj, proj_k, proj_v = np.asarray(bproj), np.asarray(proj_k), np.asarray(proj_v)

    scale = np.float32(DH ** -0.5)
    comp_a, comp_b, comp_e = ("a" in COMP), ("b" in COMP), ("e" in COMP)

    if "nc" not in _cache:
        # pick power-of-2 scales from input stats before building
        SCALES["sx"] = _pow2(96.0, np.abs(x).max())
        SCALES["spkv"] = _pow2(96.0, max(np.abs(proj_k).max(),
                                         np.abs(proj_v).max()))
        SCALES["swq"] = _pow2(96.0, float(scale) * np.abs(Wq).max())
        SCALES["swp"] = _pow2(96.0, np.abs(Wproj).max())
        SCALES["so"] = 64.0
        _cache["nc"] = build_nc()
    nc = _cache["nc"]

    projkv = np.concatenate([proj_k, proj_v], axis=1)
    if comp_a:
        xhi, xlo = _split8(x, SCALES["sx"])            # [4,4096,1024]
        x8 = np.concatenate([xhi, xlo], axis=2)        # rows [hi|lo]
        phi, plo = _split8(projkv, SCALES["spkv"])     # [4096, 512]
        pkv8 = np.ascontiguousarray(np.concatenate([plo, phi], axis=1))
    if comp_b:
        xt = np.swapaxes(x, 1, 2)                      # [4, 1024, 4096]
        thi, tlo = _split8(xt, SCALES["sx"])
        # per nb-block of 512: [lo|hi]
        t8 = np.stack([tlo.reshape(4, D, NB, DG),
                       thi.reshape(4, D, NB, DG)],
                      axis=3).reshape(4, D, 2 * N)

    in_maps = []
    for c in range(8):
        b, g = c // 2, c % 2
        cols = slice(g * DG, (g + 1) * DG)
        m = {"wk": _np_mm(Wkv[:, :D][:, cols]),
             "wv": _np_mm(Wkv[:, D:][:, cols])}
        if comp_a:
            m["x"] = np.ascontiguousarray(x8[b])
            m["projkv"] = pkv8
        else:
            m["x"] = _np_mm(x[b])
            m["projkv"] = _np_mm(projkv)
        if comp_b:
            m["xt"] = np.ascontiguousarray(t8[b])
            qhi, qlo = _split8(scale * Wq[:, cols], SCALES["swq"])
            m["wq"] = np.ascontiguousarray(
                np.concatenate([qhi, qlo], axis=1))    # rows [hi|lo]
        else:
            m["xt"] = np.ascontiguousarray(_np_mm(x[b]).T)
            m["wq"] = _np_mm(scale * Wq[:, cols])
        if comp_e:
            whi, wlo = _split8(Wproj[cols, :], SCALES["swp"])
            m["wproj"] = np.ascontiguousarray(
                np.concatenate([wlo, whi], axis=1))    # rows [lo|hi]
        else:
            m["wproj"] = _np_mm(Wproj[cols, :])
        in_maps.append(m)
    res = run_bass_kernel_spmd(nc, in_maps, list(range(8)),
                               trace=bool(os.environ.get("LINF_TRACE")))
    _cache["last_result"] = res
    outs = [np.asarray(r["out"], dtype=np.float32) for r in res.results]
    full = np.stack([outs[2 * b] + outs[2 * b + 1] for b in range(4)])
    full = full + np.asarray(bproj, np.float32)
    return full.astype(np.float32)
